# revision 1
# baseline (speedup 1.0000x reference)
"""Trainium2 Bass kernel for nn_AttentionToMotion.

Windowed multi-head attention (8 heads, 64-token windows, dim 256) with a
motion branch (attention-weighted relative coordinates through a 2-layer MLP).

kernel(**inputs) takes the FULL inputs (B=4096 windows), shards the window
batch across 8 NeuronCores (pure data parallel, 512 windows per core),
runs a Bass/Tile kernel per core, and returns the full (x, motion_out) tuple.

Self-contained: hardcodes shapes; no sibling imports.
"""
import contextlib
import os
import tempfile

import numpy as np

import concourse.bass as bass
import concourse.bacc as bacc
import concourse.tile as tile
from concourse import mybir
from concourse.bass_utils import run_bass_kernel_spmd

F32 = mybir.dt.float32
F32R = mybir.dt.float32r
BF16 = mybir.dt.bfloat16

DIM = 256
NHEADS = 8
HDIM = 32
NTOK = 64            # tokens per window
SCALE = HDIM ** -0.5
N_CORES = 8
B_FULL = 4096
NSB = 64             # superblocks (8 windows each) per core -> 512 windows/core

LAST_EXEC_TIME_NS = None


def _build_kernel(nsb, cfg=None):
    """Build the per-core Bass kernel for nsb superblocks (8 windows each)."""
    cfg = dict(cfg or {})
    sdt = cfg.get("scores_dt", F32)   # qT/kT dtype (scores matmul operands)
    edt = cfg.get("et_dt", F32)       # exp/v_aug dtype (attn@v operands)
    ntok = nsb * 512

    nc = bacc.Bacc(None, target_bir_lowering=False)
    x1 = nc.dram_tensor("x1", [ntok, DIM], F32, kind="ExternalInput")
    x2 = nc.dram_tensor("x2", [ntok, DIM], F32, kind="ExternalInput")
    wq = nc.dram_tensor("wq", [DIM, DIM], F32, kind="ExternalInput")
    wk = nc.dram_tensor("wk", [DIM, DIM], F32, kind="ExternalInput")
    wv = nc.dram_tensor("wv", [DIM, DIM], F32, kind="ExternalInput")
    wp = nc.dram_tensor("wp", [DIM, DIM], F32, kind="ExternalInput")
    bp_exp = nc.dram_tensor("bp_exp", [128, DIM], F32, kind="ExternalInput")
    identity = nc.dram_tensor("identity", [128, 128], F32, kind="ExternalInput")
    ones128 = nc.dram_tensor("ones128", [128, 128], F32, kind="ExternalInput")
    coordones = nc.dram_tensor("coordones", [128, 3], F32, kind="ExternalInput")
    wm1full = nc.dram_tensor("wm1full", [128, 64], F32, kind="ExternalInput")
    bias1exp = nc.dram_tensor("bias1exp", [128, 8], F32, kind="ExternalInput")
    wm2full = nc.dram_tensor("wm2full", [128, 8], F32, kind="ExternalInput")
    bias2exp = nc.dram_tensor("bias2exp", [128, 2], F32, kind="ExternalInput")
    yo = nc.dram_tensor("yo", [ntok, DIM], F32, kind="ExternalOutput")
    mo = nc.dram_tensor("mo", [nsb * 8, NTOK, 2], F32, kind="ExternalOutput")

    with tile.TileContext(nc) as tc:
        ctx = contextlib.ExitStack()
        consts = ctx.enter_context(tc.tile_pool(name="consts", bufs=1))
        io_pool = ctx.enter_context(tc.tile_pool(name="io", bufs=2))
        xt_pool = ctx.enter_context(tc.tile_pool(name="xt", bufs=2))
        qk_pool = ctx.enter_context(tc.tile_pool(name="qk", bufs=2))
        et_pool = ctx.enter_context(tc.tile_pool(name="et", bufs=2))
        va_pool = ctx.enter_context(tc.tile_pool(name="va", bufs=3))
        sm_pool = ctx.enter_context(tc.tile_pool(name="sm", bufs=3))
        ps_pool = ctx.enter_context(tc.tile_pool(name="ps", bufs=1, space="PSUM"))

        wq_t = consts.tile([128, 2, 2, 128], F32R, name="wq_t")
        wk_t = consts.tile([128, 2, 2, 128], F32R, name="wk_t")
        wv_t = consts.tile([128, 2, 256], F32R, name="wv_t")
        wp_t = consts.tile([128, 2, 256], F32R, name="wp_t")
        bp_t = consts.tile([128, 256], F32R, name="bp_t")
        ones_t = consts.tile([128, 128], F32R, name="ones_t")
        ident_t = consts.tile([128, 128], F32, name="ident_t")
        identr_t = consts.tile([128, 128], F32R, name="identr_t")
        co_t = consts.tile([128, 3], edt, name="co_t")
        wm1_t = consts.tile([128, 64], F32, name="wm1_t")
        b1_t = consts.tile([128, 8], F32, name="b1_t")
        wm2_t = consts.tile([128, 8], F32, name="wm2_t")
        b2_t = consts.tile([128, 2], F32, name="b2_t")
        for ch in range(2):
            for oh in range(2):
                nc.gpsimd.dma_start(out=wq_t[:, ch, oh, :], in_=wq[128 * ch:128 * ch + 128, 128 * oh:128 * oh + 128])
                nc.gpsimd.dma_start(out=wk_t[:, ch, oh, :], in_=wk[128 * ch:128 * ch + 128, 128 * oh:128 * oh + 128])
            nc.gpsimd.dma_start(out=wv_t[:, ch, :], in_=wv[128 * ch:128 * ch + 128, :])
            nc.gpsimd.dma_start(out=wp_t[:, ch, :], in_=wp[128 * ch:128 * ch + 128, :])
        nc.gpsimd.dma_start(out=bp_t[:, :], in_=bp_exp[:, :])
        nc.gpsimd.dma_start(out=ones_t[:, :], in_=ones128[:, :])
        nc.sync.dma_start(out=ident_t[:, :], in_=identity[:, :])
        nc.gpsimd.dma_start(out=identr_t[:, :], in_=identity[:, :])
        if edt == F32:
            nc.sync.dma_start(out=co_t[:, :], in_=coordones[:, :])
        else:
            nc.gpsimd.dma_start(out=co_t[:, :], in_=coordones[:, :])
        nc.sync.dma_start(out=wm1_t[:, :], in_=wm1full[:, :])
        nc.sync.dma_start(out=b1_t[:, :], in_=bias1exp[:, :])
        nc.sync.dma_start(out=wm2_t[:, :], in_=wm2full[:, :])
        nc.sync.dma_start(out=b2_t[:, :], in_=bias2exp[:, :])

        sb_banks = [ps_pool.tile([128, 512], F32, name=f"scb{r}") for r in range(4)]
        u_banks = [ps_pool.tile([128, 512], F32, name=f"ub{a}") for a in range(2)]
        proj_bank = ps_pool.tile([128, 512], F32, name="projb")
        y_bank = ps_pool.tile([128, 512], F32, name="ybk")
        tslot = [proj_bank[:, 0:128], proj_bank[:, 128:256], proj_bank[:, 256:384], proj_bank[:, 384:512]]
        pslot = [proj_bank[:, 0:256], proj_bank[:, 256:512]]
        yslot = [y_bank[:, 0:256]]
        xaslot = [y_bank[:, 256:384], y_bank[:, 384:512]]

        x1v = x1[:, :].rearrange("(s c p) k -> s p c k", p=128, c=4)
        x2v = x2[:, :].rearrange("(s c p) k -> s p c k", p=128, c=4)
        yov = yo[:, :].rearrange("(s pr p) k -> s pr p k", p=128, pr=4)

        m1_all = consts.tile([128, nsb * 4, 8], F32, name="m1_all")
        m2_all = consts.tile([128, nsb * 4, 2], F32, name="m2_all")
        pending = None

        def emit_tail(xa, s_, pr_):
            xaT = sm_pool.tile([128, 2, 128], F32R, name="xaT")
            for chf in range(2):
                slot = xaslot[chf].bitcast(F32R)
                nc.tensor.transpose(slot, xa[:, 128 * chf:128 * chf + 128], identr_t[:, :])
                if chf == 0:
                    nc.vector.tensor_copy(xaT[:, chf, :], slot)
                else:
                    nc.scalar.copy(xaT[:, chf, :], slot)
            yps = yslot[0]
            nc.tensor.matmul(yps, xaT[:, 0, :], wp_t[:, 0, :], start=True, stop=False)
            nc.tensor.matmul(yps, xaT[:, 1, :], wp_t[:, 1, :], start=False, stop=False)
            nc.tensor.matmul(yps, ones_t[:, :], bp_t[:, :], start=False, stop=True)
            y_sb = sm_pool.tile([128, 256], F32, name="y_sb")
            if pr_ % 2 == 0:
                nc.vector.tensor_copy(y_sb[:, :], yps)
            else:
                nc.scalar.copy(y_sb[:, :], yps)
            nc.sync.dma_start(out=yov[s_, pr_], in_=y_sb[:, :])

        for s in range(nsb):
            x1s = io_pool.tile([128, 4, 256], F32R, name="x1s")
            x2s = io_pool.tile([128, 4, 256], F32R, name="x2s")
            nc.gpsimd.dma_start(out=x1s[:, :, :], in_=x1v[s])
            nc.gpsimd.dma_start(out=x2s[:, :, :], in_=x2v[s])

            x1T = xt_pool.tile([128, 2, 512], F32R, name="x1T")
            x2T = xt_pool.tile([128, 2, 512], F32R, name="x2T")
            g = 0
            for (src, dstT) in ((x1s, x1T), (x2s, x2T)):
                for chf in range(2):
                    for p4g in range(2):
                        base = 2 * (g % 2)
                        for j in range(2):
                            p4 = 2 * p4g + j
                            nc.tensor.transpose(tslot[base + j].bitcast(F32R),
                                                src[:, p4, 128 * chf:128 * chf + 128],
                                                identr_t[:, :])
                        src_ap = proj_bank[:, 256 * (g % 2):256 * (g % 2) + 256].bitcast(F32R)
                        if g % 2 == 0:
                            nc.vector.tensor_copy(dstT[:, chf, 256 * p4g:256 * p4g + 256], src_ap)
                        else:
                            nc.scalar.copy(dstT[:, chf, 256 * p4g:256 * p4g + 256], src_ap)
                        g += 1

            qT = qk_pool.tile([128, 2, 512], sdt, name="qT")
            kT = qk_pool.tile([128, 2, 512], sdt, name="kT")
            pi = 0
            for (wt, xT, dst) in ((wq_t, x1T, qT), (wk_t, x2T, kT)):
                for oh in range(2):
                    for tchunk in range(2):
                        slot = pslot[tchunk]
                        for ch in range(2):
                            nc.tensor.matmul(slot, wt[:, ch, oh, :],
                                             xT[:, ch, 256 * tchunk:256 * tchunk + 256],
                                             start=(ch == 0), stop=(ch == 1))
                    if pi % 2 == 0:
                        nc.vector.tensor_copy(dst[:, oh, :], proj_bank[:, :])
                    else:
                        nc.scalar.copy(dst[:, oh, :], proj_bank[:, :])
                    pi += 1

            ET = [et_pool.tile([128, 512], edt, name=f"ET{r}") for r in range(4)]
            for pr in range(4):
                vps = pslot[pr % 2]
                for ch in range(2):
                    nc.tensor.matmul(vps, x2T[:, ch, 128 * pr:128 * pr + 128], wv_t[:, ch, :],
                                     start=(ch == 0), stop=(ch == 1))
                v_aug = va_pool.tile([128, 8, 35], edt, name="v_aug")
                va_v = v_aug[:, :, :]
                nc.vector.tensor_copy(va_v[:, :, 0:32], vps.rearrange("p (h d) -> p h d", h=8))
                nc.vector.tensor_copy(va_v[:, :, 32:35],
                                      co_t[:, :].unsqueeze(1).broadcast_to((128, 8, 3)))

                foff = 128 * pr
                for a in range(2):
                    wl = 2 * pr + a
                    for h in range(8):
                        r = h % 4
                        hs = h // 4
                        out = sb_banks[r][64 * a:64 * a + 64, foff + 64 * hs: foff + 64 * hs + 64]
                        nc.tensor.matmul(out,
                                         kT[32 * r:32 * r + 32, hs, 64 * wl:64 * wl + 64],
                                         qT[32 * r:32 * r + 32, hs, 64 * wl:64 * wl + 64],
                                         start=True, stop=True,
                                         tile_position=(32 * r, 64 * a))
                for r in range(4):
                    nc.scalar.activation(ET[r][:, foff:foff + 128],
                                         sb_banks[r][:, foff:foff + 128],
                                         mybir.ActivationFunctionType.Exp,
                                         bias=0.0, scale=SCALE)
                for a in range(2):
                    for h in range(8):
                        r = h % 4
                        hs = h // 4
                        lhs = ET[r][64 * a:64 * a + 64, foff + 64 * hs: foff + 64 * hs + 64]
                        rhs = v_aug[64 * a:64 * a + 64, h, 0:35]
                        out = u_banks[a][64 * a:64 * a + 64, 35 * h:35 * h + 35]
                        nc.tensor.matmul(out, lhs, rhs, start=True, stop=True,
                                         tile_position=(64 * a, 64 * a))
                rz = sm_pool.tile([128, 8], F32, name="rz")
                xa = sm_pool.tile([128, 256], F32R, name="xa")
                xm = sm_pool.tile([128, 8, 2], F32, name="xm")
                for a in range(2):
                    u = u_banks[a][64 * a:64 * a + 64, 0:280].rearrange("p (h e) -> p h e", e=35)
                    nc.vector.reciprocal(rz[64 * a:64 * a + 64, :], u[:, :, 34])
                    rzs = rz[64 * a:64 * a + 64, :]
                    nc.vector.tensor_mul(
                        xa[64 * a:64 * a + 64, :].rearrange("p (h d) -> p h d", d=32),
                        u[:, :, 0:32],
                        rzs.unsqueeze(2).broadcast_to((64, 8, 32)))
                    nc.vector.tensor_mul(
                        xm[64 * a:64 * a + 64, :, :],
                        u[:, :, 32:34],
                        rzs.unsqueeze(2).broadcast_to((64, 8, 2)))

                if pending is not None:
                    emit_tail(*pending)
                pending = (xa, s, pr)

                t1 = sm_pool.tile([128, 64], F32, name="t1")
                nc.vector.tensor_mul(
                    t1[:, :].rearrange("p (h c j) -> p h c j", h=8, c=2),
                    xm[:, :, :].unsqueeze(3).broadcast_to((128, 8, 2, 4)),
                    wm1_t[:, :].rearrange("p (h c j) -> p h c j", h=8, c=2))
                pidx = s * 4 + pr
                nc.vector.reduce_sum(m1_all[:, pidx, :].rearrange("p (c j) -> p c j", c=2),
                                     t1[:, :].rearrange("p (h c j) -> p c j h", h=8, c=2),
                                     axis=mybir.AxisListType.X)

        if pending is not None:
            emit_tail(*pending)
            pending = None

        npair = nsb * 4
        CH = 64
        for c0 in range(0, npair, CH):
            cn = min(CH, npair - c0)
            m1c = m1_all[:, c0:c0 + cn, :]
            nc.vector.tensor_add(m1c, m1c,
                                 b1_t[:, :].unsqueeze(1).broadcast_to((128, cn, 8)))
            g1 = sm_pool.tile([128, CH, 8], F32, name="g1")
            nc.scalar.activation(g1[:, 0:cn, :], m1c,
                                 mybir.ActivationFunctionType.Gelu, bias=0.0, scale=1.0)
            nc.vector.tensor_mul(g1[:, 0:cn, :], g1[:, 0:cn, :],
                                 wm2_t[:, :].unsqueeze(1).broadcast_to((128, cn, 8)))
            nc.vector.reduce_sum(m2_all[:, c0:c0 + cn, :],
                                 g1[:, 0:cn, :].rearrange("p s (c j) -> p s c j", c=2),
                                 axis=mybir.AxisListType.X)
            nc.vector.tensor_add(m2_all[:, c0:c0 + cn, :], m2_all[:, c0:c0 + cn, :],
                                 b2_t[:, :].unsqueeze(1).broadcast_to((128, cn, 2)))
        nc.sync.dma_start(out=mo[:, :, :].rearrange("(sp a) n c -> (a n) sp c", a=2),
                          in_=m2_all[:, :, :])
        ctx.close()
    nc.finalize()
    return nc


def _host_consts(Wq, Wkv, Wproj, bproj, Wm1, bm1, Wm2, bm2):
    col = (np.arange(NTOK) % 8).astype(np.float32)
    row = (np.arange(NTOK) // 8).astype(np.float32)
    coordones = np.stack([np.concatenate([col, col]), np.concatenate([row, row]),
                          np.ones(128, np.float32)], axis=1).astype(np.float32)
    wm1full = np.ascontiguousarray(
        np.broadcast_to(Wm1[None, :, None, :], (128, 8, 2, 4)).reshape(128, 64)).astype(np.float32)
    wm1sum = Wm1.sum(axis=0)
    coord_n = np.stack([col, row], axis=0)
    b1 = bm1[None, None, :] - coord_n[:, :, None] * wm1sum[None, None, :]
    b1 = b1.transpose(1, 0, 2).reshape(64, 8)
    bias1exp = np.concatenate([b1, b1], axis=0).astype(np.float32)
    wm2full = np.ascontiguousarray(
        np.broadcast_to(np.asarray(Wm2).reshape(-1)[None, None, :], (128, 2, 4)).reshape(128, 8)).astype(np.float32)
    bias2exp = np.full((128, 2), float(np.asarray(bm2).reshape(-1)[0]), np.float32)
    bp_expa = np.zeros((128, DIM), np.float32)
    bp_expa[0, :] = bproj
    return {
        "wq": np.ascontiguousarray(Wq.astype(np.float32)),
        "wk": np.ascontiguousarray(Wkv[:, 0:256].astype(np.float32)),
        "wv": np.ascontiguousarray(Wkv[:, 256:512].astype(np.float32)),
        "wp": np.ascontiguousarray(Wproj.astype(np.float32)),
        "bp_exp": bp_expa,
        "identity": np.eye(128, dtype=np.float32),
        "ones128": np.ones((128, 128), np.float32),
        "coordones": coordones,
        "wm1full": wm1full,
        "bias1exp": bias1exp,
        "wm2full": wm2full,
        "bias2exp": bias2exp,
    }


_NC_CACHE = {}


def kernel(x1, x2, H=None, W=None, Wq=None, Wkv=None, Wproj=None, bproj=None,
           Wm1=None, bm1=None, Wm2=None, bm2=None, **_ignored):
    global LAST_EXEC_TIME_NS
    x1 = np.asarray(x1, dtype=np.float32)
    x2 = np.asarray(x2, dtype=np.float32)
    B, n, C = x1.shape
    assert (B, n, C) == (B_FULL, NTOK, DIM), (B, n, C)
    Wq = np.asarray(Wq, np.float32)
    Wkv = np.asarray(Wkv, np.float32)
    Wproj = np.asarray(Wproj, np.float32)
    bproj = np.asarray(bproj, np.float32)
    Wm1 = np.asarray(Wm1, np.float32)
    bm1 = np.asarray(bm1, np.float32)
    Wm2 = np.asarray(Wm2, np.float32)
    bm2 = np.asarray(bm2, np.float32)

    cfg_name = os.environ.get("ATTN_DTYPE", "f32r")
    cfg = {} if cfg_name == "f32r" else {"scores_dt": BF16, "et_dt": BF16}
    key = (NSB, cfg_name)
    if key not in _NC_CACHE:
        _NC_CACHE[key] = _build_kernel(NSB, cfg)
    nc = _NC_CACHE[key]

    consts = _host_consts(Wq, Wkv, Wproj, bproj, Wm1, bm1, Wm2, bm2)
    wpc = B // N_CORES  # windows per core
    in_maps = []
    for c in range(N_CORES):
        m = dict(consts)
        m["x1"] = np.ascontiguousarray(x1[c * wpc:(c + 1) * wpc].reshape(wpc * NTOK, DIM))
        m["x2"] = np.ascontiguousarray(x2[c * wpc:(c + 1) * wpc].reshape(wpc * NTOK, DIM))
        in_maps.append(m)

    trace = os.environ.get("ATTN_TRACE", "0") == "1"
    kw = {}
    if trace:
        kw = dict(trace=True, tmpdir=tempfile.mkdtemp(prefix="attnkern"))
    res = run_bass_kernel_spmd(nc, in_maps, core_ids=list(range(N_CORES)), **kw)
    LAST_EXEC_TIME_NS = res.exec_time_ns

    x_out = np.empty((B, NTOK, DIM), np.float32)
    m_out = np.empty((B, NTOK, 2), np.float32)
    for c in range(N_CORES):
        x_out[c * wpc:(c + 1) * wpc] = res.results[c]["yo"].reshape(wpc, NTOK, DIM)
        m_out[c * wpc:(c + 1) * wpc] = res.results[c]["mo"]
    return (x_out, m_out)
